# revision 38
# baseline (speedup 1.0000x reference)
"""Trainium2 Bass kernel for GQA attention (B=1, S=2048, D=2048, H=32, KV=8, HD=64).

Tensor-parallel over heads across 8 NeuronCores: core i holds q-heads
[4i, 4i+4) and kv-head i; each core computes its partial o_proj output and the
host sums the 8 partials (Megatron all-reduce done host-side).

Schedule: attention units (head h, q-block qb) are interleaved into the
projection phase as soon as their inputs exist (unit (h,qb) only needs
s-blocks 0..qb projected), so the ACT engine (exp is the per-unit bottleneck)
is busy from early on while the PE does projections. o_proj is released
per-q-block as soon as all 4 heads of that block are normalized.

Self-contained: only imports concourse (on sys.path in the container).
"""

import os
import sys

import ml_dtypes
import numpy as np

if "/opt/trn_rl_repo" not in sys.path and not any(
    p.endswith("trn_rl_repo") for p in sys.path
):
    sys.path.insert(0, "/opt/trn_rl_repo")

import concourse.bass as bass
import concourse.mybir as mybir
import concourse.tile as tile
from concourse import bacc
from concourse.bass_utils import run_bass_kernel_spmd
from concourse.masks import make_identity

F32 = mybir.dt.float32
BF16 = mybir.dt.bfloat16

AF = mybir.ActivationFunctionType
ALU = mybir.AluOpType

S = 2048
D = 2048
H = 32
KV = 8
HD = 64
NCORES = 8
HQ = H // NCORES  # 4 q heads per core
NKC = S // 128  # 16 key chunks
NQB = 4  # q blocks of 512
QBW = 512
NSB = 4  # s blocks of 512 in projection
SBW = 512
DCH = D // 128  # 16 contraction chunks


def _build_nc():
    nc = bacc.Bacc("TRN2", target_bir_lowering=False, debug=False, num_devices=NCORES)

    xt_d = nc.declare_dram_parameter("xt", [D, S], BF16, isOutput=False)
    wqkv_d = nc.declare_dram_parameter("wqkv", [D, 384], BF16, isOutput=False)
    wo_d = nc.declare_dram_parameter("wo", [2, 128, D], BF16, isOutput=False)
    cos_d = nc.declare_dram_parameter("cos", [128, S], BF16, isOutput=False)
    sin_d = nc.declare_dram_parameter("sin", [128, S], BF16, isOutput=False)
    sel_d = nc.declare_dram_parameter("sel", [4, 2 * 128], BF16, isOutput=False)
    y_d = nc.declare_dram_parameter("y", [S, D], BF16, isOutput=True)

    with tile.TileContext(nc) as tc:
        with (
            tc.tile_pool(name="glob", bufs=1) as glob,
        ):
            ktdup = glob.tile([128, S], BF16, tag="ktdup")
            v_s = glob.tile([128, NKC, 65], BF16, tag="v_s")
            outA = glob.tile([128, S], BF16, tag="outA")
            outB = glob.tile([128, S], BF16, tag="outB")
            ao = glob.tile([128, 2, S], BF16, tag="ao")
            sel_s = glob.tile([4, 2 * 128], BF16, tag="sel_s")
            wo_s = glob.tile([128, 2, D], BF16, tag="wo_s")
            # per-qb sum tiles: custom-DVE ops (reciprocal) need partition
            # base 0, so each q-block gets its own 4-row tile (row = head)
            sums_qb = [
                glob.tile([4, QBW], F32, tag="sums", name=f"sums{i}")
                for i in range(NQB)
            ]
            rcp_f32 = glob.tile([4, QBW], F32, tag="rcp_f32")
            rcp_bf = glob.tile([4, QBW], BF16, tag="rcp_bf")
            rcp_scr = glob.tile([4, QBW], F32, tag="rcp_scr")
            ident = glob.tile([128, 128], F32, tag="ident")
            warm = glob.tile([1, 16], F32, tag="warm")

            nc.vector.memset(v_s[:, :, 64], 1.0)
            for t in sums_qb:
                nc.vector.memset(t[:], 1.0)
            # Preload the Exp table set during the initial DMA wait.
            nc.vector.memset(warm[:], 0.0)
            nc.scalar.activation(warm[:], warm[:], AF.Exp)

            with (
                tc.tile_pool(name="p1", bufs=1) as p1,
                tc.tile_pool(name="xp", bufs=3) as xp,
                tc.tile_pool(name="tmpp", bufs=4) as tmpp,
                tc.tile_pool(name="qsp", bufs=4) as qsp,
                tc.tile_pool(name="ptp", bufs=8) as ptp,
                tc.tile_pool(name="stgp", bufs=4) as stgp,
                tc.tile_pool(name="yp", bufs=8) as yp,
                tc.tile_pool(name="ps1", bufs=2, space="PSUM") as ps1,
                tc.tile_pool(name="pssc", bufs=2, space="PSUM") as pssc,
                tc.tile_pool(name="pso_p", bufs=2, space="PSUM") as pso_p,
            ):
                wq_s = p1.tile([128, DCH, 384], BF16, tag="wq_s")
                wqkv_r = wqkv_d.rearrange("(ko p) n -> p ko n", p=128)
                cos_s = p1.tile([128, S], BF16, tag="cos_s")
                sin_s = p1.tile([128, S], BF16, tag="sin_s")
                kvraw = p1.tile([128, S], F32, tag="kvraw")
                kswap = p1.tile([64, S], F32, tag="kswap")

                qs_all = [
                    qsp.tile([128, S], BF16, tag="qs", name=f"qs{i}") for i in range(HQ)
                ]

                def stream_qs(h, qb):
                    """Stage head-h q data (duplicated per array-half) for block qb."""
                    hc = slice(32 * h, 32 * h + 32)
                    qc = slice(qb * QBW, (qb + 1) * QBW)
                    qs = qs_all[h]
                    nc.sync.dma_start(qs[0:32, qc], outA[hc, qc])
                    nc.sync.dma_start(qs[32:64, qc], outB[hc, qc])
                    nc.sync.dma_start(qs[64:96, qc], outA[hc, qc])
                    nc.sync.dma_start(qs[96:128, qc], outB[hc, qc])

                def unit_gen(h, qb):
                    """Scores + softmax-exp + PV for (head h, q-block qb),
                    yielding after each chunk-pair so the caller can interleave
                    pair emission with projection-chain segments (keeps the
                    exp pipeline fed while long PE chains run).

                    Diagonal chunks (kc0 >= q0) only compute/consume the causal
                    q-range [kc0, q0+512): d = kc0-q0 cols are skipped in the
                    scores MM, exp, select and PV.  The one exception: when
                    d == 128 (second chunk of the second-to-last pair) the
                    scores MM computes from 0 anyway so a single exp can span
                    [dA:1024] without reading unwritten PSUM; the extra cols
                    are never consumed downstream.
                    """
                    qs = qs_all[h]
                    q0 = qb * QBW
                    nkc = 4 * (qb + 1)
                    pso = pso_p.tile([128, QBW], F32, tag="pso")
                    for pair in range(nkc // 2):
                        yield
                        cA, cB = 2 * pair, 2 * pair + 1
                        psc = pssc.tile([128, 1024], F32, tag="psc")
                        ptt = ptp.tile([128, 1024], BF16, tag="ptt")
                        dA = max(0, cA * 128 - q0)
                        dB = max(0, cB * 128 - q0)
                        for c, half, r0, d in ((cA, 0, 0, dA), (cB, 1, 64, dB)):
                            dm = 0 if d == 128 else d
                            nc.tensor.matmul(
                                psc[:, half * 512 + dm : half * 512 + 512],
                                lhsT=ktdup[r0 : r0 + 64, c * 128 : c * 128 + 128],
                                rhs=qs[r0 : r0 + 64, q0 + dm : q0 + QBW],
                                start=True,
                                stop=True,
                                tile_position=(r0, 0),
                            )
                        if dB > 128:
                            nc.scalar.activation(
                                ptt[:, dA:512], psc[:, dA:512], AF.Exp
                            )
                            nc.scalar.activation(
                                ptt[:, 512 + dB : 1024], psc[:, 512 + dB : 1024],
                                AF.Exp,
                            )
                        else:
                            nc.scalar.activation(
                                ptt[:, dA:1024], psc[:, dA:1024], AF.Exp
                            )
                        for c, half, d in ((cA, 0, dA), (cB, 1, dB)):
                            kc0 = c * 128
                            if kc0 >= q0:
                                ww = min(128, 512 - d)
                                s0 = half * 512 + d
                                nc.gpsimd.affine_select(
                                    out=ptt[:, s0 : s0 + ww],
                                    in_=ptt[:, s0 : s0 + ww],
                                    compare_op=ALU.is_ge,
                                    fill=0.0,
                                    base=0,
                                    channel_multiplier=-1,
                                    pattern=[[1, ww]],
                                )
                        for c, half, d in ((cA, 0, dA), (cB, 1, dB)):
                            nc.tensor.matmul(
                                pso[0:65, d:QBW],
                                lhsT=v_s[:, c, :],
                                rhs=ptt[:, half * 512 + d : half * 512 + 512],
                                start=(c == 0),
                                stop=(c == nkc - 1),
                            )
                    # evict raw attn out (rows 0-63) + exp-sum (row 64)
                    ch = h // 2
                    rr = 64 * (h % 2)
                    qc = slice(q0, q0 + QBW)
                    if rr == 0:
                        nc.vector.tensor_copy(ao[0:64, ch, qc], pso[0:64, :])
                    else:
                        stg = stgp.tile([64, QBW], BF16, tag="stg")
                        nc.vector.tensor_copy(stg[:], pso[0:64, :])
                        nc.sync.dma_start(ao[64:128, ch, qc], stg[:])
                    sumr = stgp.tile([1, QBW], F32, tag="sumr")
                    nc.vector.tensor_copy(sumr[:], pso[64:65, :])
                    nc.gpsimd.dma_start(sums_qb[qb][h : h + 1, :], sumr[:])

                pending = []  # FIFO of [qb, unit generator]
                _rr = [0]

                def pump(n):
                    """Advance up to n pair-steps, round-robin over the two
                    oldest pending units (two independent score/exp/PV streams
                    hide each other's latency)."""
                    done = 0
                    while pending and done < n:
                        k = _rr[0] % min(2, len(pending))
                        try:
                            next(pending[k][1])
                            done += 1
                            _rr[0] += 1
                        except StopIteration:
                            pending.pop(k)

                def drain_qb(qb):
                    """Fully emit every pending unit of q-blocks <= qb."""
                    i = 0
                    while i < len(pending):
                        if pending[i][0] <= qb:
                            for _ in pending[i][1]:
                                pass
                            pending.pop(i)
                        else:
                            i += 1

                def norm_oproj(qb):
                    """Normalize all heads for q-block qb, then o_proj its rows."""
                    q0 = qb * QBW
                    qc = slice(q0, q0 + QBW)
                    nc.vector.reciprocal_approx_accurate(
                        rcp_f32[:], sums_qb[qb][:], rcp_scr[:]
                    )
                    nc.vector.tensor_copy(rcp_bf[:], rcp_f32[:])
                    for ch in range(2):
                        pbc = ps1.tile([128, QBW], F32, tag="proj")
                        nc.tensor.matmul(
                            pbc[:],
                            lhsT=sel_s[:, ch * 128 : (ch + 1) * 128],
                            rhs=rcp_bf[:],
                            start=True,
                            stop=True,
                        )
                        nc.vector.tensor_tensor(
                            ao[:, ch, qc], ao[:, ch, qc], pbc[:], ALU.mult
                        )
                    for st in range(4 * qb, 4 * qb + 4):
                        for obp in range(2):
                            if qb == 3 and obp == 1:
                                # exp stream is over: use the freed score-PSUM
                                # banks to deepen the o_proj pipeline
                                psys = [
                                    pssc.tile(
                                        [128, 1024], F32, tag="psc", name=f"psy{oh}"
                                    )[:, 0:QBW]
                                    for oh in range(2)
                                ]
                            else:
                                psys = [
                                    ps1.tile(
                                        [128, QBW], F32, tag="proj", name=f"psy{oh}"
                                    )
                                    for oh in range(2)
                                ]
                            for chp in range(2):
                                for oh in range(2):
                                    ob = 2 * obp + oh
                                    nc.tensor.matmul(
                                        psys[oh][:],
                                        lhsT=ao[:, chp, st * 128 : (st + 1) * 128],
                                        rhs=wo_s[:, chp, ob * 512 : (ob + 1) * 512],
                                        start=(chp == 0),
                                        stop=(chp == 1),
                                    )
                            for oh in range(2):
                                ob = 2 * obp + oh
                                ysb = yp.tile([128, QBW], BF16, tag="ysb")
                                if qb == 3 and oh == 1:
                                    nc.scalar.activation(ysb[:], psys[oh][:], AF.Copy)
                                else:
                                    nc.vector.tensor_copy(ysb[:], psys[oh][:])
                                eng = nc.gpsimd if (st + ob) % 2 == 0 else nc.sync
                                eng.dma_start(
                                    y_d[
                                        st * 128 : (st + 1) * 128,
                                        ob * 512 : (ob + 1) * 512,
                                    ],
                                    ysb[:],
                                )

                # units become available after projection s-block sb (unit
                # (h, qb) needs s-blocks 0..qb); their chunk-pairs are pumped
                # one at a time between projection-chain segments
                unit_sched = {
                    0: [(0, 0), (1, 0), (2, 0), (3, 0)],
                    1: [(0, 1), (1, 1), (2, 1), (3, 1)],
                    2: [(0, 2), (1, 2), (2, 2), (3, 2)],
                    3: [(0, 3), (1, 3), (2, 3), (3, 3)],
                }

                xt_r = xt_d.rearrange("(ko p) s -> p ko s", p=128)
                for sb in range(NSB):
                    sbc = slice(sb * SBW, (sb + 1) * SBW)
                    xblk = xp.tile([128, DCH, SBW], BF16, tag="xblk")
                    for kq in range(4):
                        if sb == 0:
                            # weights stream on the gpsimd queue so they land
                            # in parallel with the x blocks on sync
                            for kc in range(4 * kq, 4 * kq + 4):
                                nc.gpsimd.dma_start(wq_s[:, kc, :], wqkv_r[:, kc, :])
                        nc.sync.dma_start(
                            xblk[:, 4 * kq : 4 * kq + 4, :],
                            xt_r[:, 4 * kq : 4 * kq + 4, sbc],
                        )
                    if sb == 0:
                        nc.gpsimd.dma_start(cos_s[:], cos_d[:])
                        nc.gpsimd.dma_start(sin_s[:], sin_d[:])
                        make_identity(nc, ident[:])
                    if sb == 1:
                        nc.gpsimd.dma_start(sel_s[:], sel_d[:])
                        for ch in range(2):
                            nc.gpsimd.dma_start(wo_s[:, ch, :], wo_d[ch])
                    # A/B chains first: the KV chain then overlaps the psA/psB
                    # eviction-by-RoPE latency, and the KV eviction itself only
                    # gates the NEXT s-block's pairs
                    psA = ps1.tile([128, SBW], F32, tag="proj")
                    psB = ps1.tile([128, SBW], F32, tag="proj")
                    psKV = ps1.tile([128, SBW], F32, tag="proj")
                    for ps_t, col0 in ((psA, 0), (psB, 128), (psKV, 256)):
                        for kq in range(4):
                            for kc in range(4 * kq, 4 * kq + 4):
                                nc.tensor.matmul(
                                    ps_t[:],
                                    lhsT=wq_s[:, kc, col0 : col0 + 128],
                                    rhs=xblk[:, kc, :],
                                    start=(kc == 0),
                                    stop=(kc == DCH - 1),
                                )
                            pump(1)
                    # evict k|v rows early (frees the KV slot); DVE keeps the
                    # scalar engine free for the exp stream
                    nc.vector.tensor_copy(kvraw[:, sbc], psKV[:])

                    # RoPE on the 4 q heads (A = first-half dims, B = second)
                    tmp = tmpp.tile([128, SBW], F32, tag="tmp")
                    nc.vector.tensor_tensor(
                        outA[:, sbc], psA[:], cos_s[:, sbc], ALU.mult
                    )
                    nc.vector.tensor_tensor(tmp[:], psB[:], sin_s[:, sbc], ALU.mult)
                    nc.vector.tensor_tensor(
                        outA[:, sbc], outA[:, sbc], tmp[:], ALU.subtract
                    )
                    tmp2 = tmpp.tile([128, SBW], F32, tag="tmp")
                    nc.vector.tensor_tensor(
                        outB[:, sbc], psB[:], cos_s[:, sbc], ALU.mult
                    )
                    nc.vector.tensor_tensor(tmp2[:], psA[:], sin_s[:, sbc], ALU.mult)
                    nc.vector.tensor_tensor(
                        outB[:, sbc], outB[:, sbc], tmp2[:], ALU.add
                    )

                    # k RoPE on this s-block: kswap = [k_hi; k_lo]
                    nc.gpsimd.dma_start(kswap[0:32, sbc], kvraw[32:64, sbc])
                    nc.gpsimd.dma_start(kswap[32:64, sbc], kvraw[0:32, sbc])
                    nc.vector.tensor_tensor(
                        ktdup[0:64, sbc], kvraw[0:64, sbc], cos_s[0:64, sbc], ALU.mult
                    )
                    tmpk = tmpp.tile([64, SBW], F32, tag="tmpk")
                    nc.vector.tensor_tensor(
                        tmpk[:], kswap[:, sbc], sin_s[0:64, sbc], ALU.mult
                    )
                    nc.vector.tensor_tensor(
                        ktdup[0:32, sbc], ktdup[0:32, sbc], tmpk[0:32, :],
                        ALU.subtract,
                    )
                    nc.vector.tensor_tensor(
                        ktdup[32:64, sbc], ktdup[32:64, sbc], tmpk[32:64, :],
                        ALU.add,
                    )
                    nc.gpsimd.dma_start(ktdup[64:128, sbc], ktdup[0:64, sbc])

                    # v: [64, 512] -> 4 key-chunk tiles [128, 64] via PE transpose
                    for c in range(4 * sb, 4 * sb + 4):
                        ptr = pso_p.tile([128, QBW], F32, tag="pso")
                        nc.tensor.transpose(
                            ptr[:, 0:64],
                            kvraw[64:128, c * 128 : (c + 1) * 128],
                            ident[64:128, 64:128],
                        )
                        nc.vector.tensor_copy(v_s[:, c, 0:64], ptr[:, 0:64])
                        pump(1)

                    for h, qb in unit_sched[sb]:
                        stream_qs(h, qb)
                        pending.append([qb, unit_gen(h, qb)])
                    pump(8 if sb == 0 else 3)

                # tail: remaining unit pairs interleaved with per-block
                # norm + o_proj (norm_oproj(qb) only needs units (*, qb), so
                # its PE work overlaps the still-streaming exp pipeline)
                for qb in range(NQB):
                    drain_qb(qb)
                    norm_oproj(qb)
                    pump(6)
    nc.compile()
    return nc


def _prep_inputs(x, Wq, Wk, Wv, Wo, inv_freq):
    """Host-side sharding + layout prep. Returns in_maps for the 8 cores."""
    x = np.ascontiguousarray(np.asarray(x, dtype=np.float32).reshape(S, D))
    xt = np.ascontiguousarray(x.T)  # [D, S]

    pos = np.arange(S, dtype=np.float64)
    inv = np.asarray(inv_freq, dtype=np.float64)  # [32]
    freqs = pos[None, :] * inv[:, None]  # [32, S]
    cos32 = np.cos(freqs).astype(np.float32)
    sin32 = np.sin(freqs).astype(np.float32)
    cos_tab = np.tile(cos32, (4, 1))  # [128, S]
    sin_tab = np.tile(sin32, (4, 1))
    # sel[h, ch*128 + r]: broadcast rcp row (head) 2ch into output rows 0-63
    # and 2ch+1 into rows 64-127 (same pattern for every q-block).
    sel = np.zeros((4, 2 * 128), dtype=np.float32)
    for ch in range(2):
        sel[2 * ch, ch * 128 : ch * 128 + 64] = 1.0
        sel[2 * ch + 1, ch * 128 + 64 : ch * 128 + 128] = 1.0

    in_maps = []
    for i in range(NCORES):
        wq_l = Wq[256 * i : 256 * (i + 1)].astype(np.float32) * 0.125  # [256, D]
        wk_l = Wk[64 * i : 64 * (i + 1)].astype(np.float32)  # [64, D]
        wv_l = Wv[64 * i : 64 * (i + 1)].astype(np.float32)  # [64, D]
        # A-tile: first-half dims of the 4 heads; B-tile: second halves
        wA = np.concatenate(
            [wq_l[64 * h : 64 * h + 32] for h in range(HQ)], axis=0
        )  # [128, D]
        wB = np.concatenate(
            [wq_l[64 * h + 32 : 64 * h + 64] for h in range(HQ)], axis=0
        )
        wkv = np.concatenate([wk_l, wv_l], axis=0)  # [128, D]
        wqkv = np.ascontiguousarray(
            np.concatenate([wA, wB, wkv], axis=0).T
        )  # [D, 384]
        wo_l = Wo[:, 256 * i : 256 * (i + 1)].astype(np.float32)  # [D, 256]
        wo_t = np.ascontiguousarray(wo_l.T.reshape(2, 128, D))  # [2, 128, D]
        in_maps.append(
            {
                "xt": xt.astype(ml_dtypes.bfloat16),
                "wqkv": wqkv.astype(ml_dtypes.bfloat16),
                "wo": wo_t.astype(ml_dtypes.bfloat16),
                "cos": cos_tab.astype(ml_dtypes.bfloat16),
                "sin": sin_tab.astype(ml_dtypes.bfloat16),
                "sel": sel.astype(ml_dtypes.bfloat16),
            }
        )
    return in_maps


_NC_CACHE = None


def kernel(x, Wq, Wk, Wv, Wo, inv_freq):
    global _NC_CACHE
    if _NC_CACHE is None:
        _NC_CACHE = _build_nc()
    nc = _NC_CACHE
    in_maps = _prep_inputs(x, Wq, Wk, Wv, Wo, inv_freq)
    trace = bool(int(os.environ.get("BASS_KERNEL_TRACE", "0")))
    res = None
    last_exc = None
    for attempt in range(3):
        try:
            res = run_bass_kernel_spmd(nc, in_maps, list(range(NCORES)), trace=trace)
            break
        except Exception as e:  # transient device faults (rare) — retry
            last_exc = e
            msg = str(e)
            if "UNRECOVERABLE" in msg or "UNAVAILABLE" in msg or "Timeout" in msg:
                continue
            raise
    if res is None:
        raise last_exc
    if trace:
        kernel.last_results = res
    y = np.zeros((S, D), dtype=np.float32)
    for i in range(NCORES):
        y += res.results[i]["y"].astype(np.float32)
    return y.reshape(1, S, D)


# revision 40
# speedup vs baseline: 1.0503x; 1.0503x over previous
"""Trainium2 Bass kernel for GQA attention (B=1, S=2048, D=2048, H=32, KV=8, HD=64).

Tensor-parallel over heads across 8 NeuronCores: core i holds q-heads
[4i, 4i+4) and kv-head i; each core computes its partial o_proj output and the
host sums the 8 partials (Megatron all-reduce done host-side).

Schedule: attention units (head h, q-block qb) are interleaved into the
projection phase as soon as their inputs exist (unit (h,qb) only needs
s-blocks 0..qb projected), so the ACT engine (exp is the per-unit bottleneck)
is busy from early on while the PE does projections. o_proj is released
per-q-block as soon as all 4 heads of that block are normalized.

Self-contained: only imports concourse (on sys.path in the container).
"""

import os
import sys

import ml_dtypes
import numpy as np

if "/opt/trn_rl_repo" not in sys.path and not any(
    p.endswith("trn_rl_repo") for p in sys.path
):
    sys.path.insert(0, "/opt/trn_rl_repo")

import concourse.bass as bass
import concourse.mybir as mybir
import concourse.tile as tile
from concourse import bacc
from concourse.bass_utils import run_bass_kernel_spmd
from concourse.masks import make_identity

F32 = mybir.dt.float32
BF16 = mybir.dt.bfloat16

AF = mybir.ActivationFunctionType
ALU = mybir.AluOpType

S = 2048
D = 2048
H = 32
KV = 8
HD = 64
NCORES = 8
HQ = H // NCORES  # 4 q heads per core
NKC = S // 128  # 16 key chunks
NQB = 4  # q blocks of 512
QBW = 512
NSB = 4  # s blocks of 512 in projection
SBW = 512
DCH = D // 128  # 16 contraction chunks


def _build_nc():
    nc = bacc.Bacc("TRN2", target_bir_lowering=False, debug=False, num_devices=NCORES)

    xt_d = nc.declare_dram_parameter("xt", [D, S], BF16, isOutput=False)
    wqkv_d = nc.declare_dram_parameter("wqkv", [D, 384], BF16, isOutput=False)
    wo_d = nc.declare_dram_parameter("wo", [2, 128, D], BF16, isOutput=False)
    cos_d = nc.declare_dram_parameter("cos", [128, S], BF16, isOutput=False)
    sin_d = nc.declare_dram_parameter("sin", [128, S], BF16, isOutput=False)
    sel_d = nc.declare_dram_parameter("sel", [4, 2 * 128], BF16, isOutput=False)
    y_d = nc.declare_dram_parameter("y", [S, D], BF16, isOutput=True)

    with tile.TileContext(nc) as tc:
        with (
            tc.tile_pool(name="glob", bufs=1) as glob,
        ):
            ktdup = glob.tile([128, S], BF16, tag="ktdup")
            v_s = glob.tile([128, NKC, 65], BF16, tag="v_s")
            outA = glob.tile([128, S], BF16, tag="outA")
            outB = glob.tile([128, S], BF16, tag="outB")
            ao = glob.tile([128, 2, S], BF16, tag="ao")
            sel_s = glob.tile([4, 2 * 128], BF16, tag="sel_s")
            wo_s = glob.tile([128, 2, D], BF16, tag="wo_s")
            # per-qb sum tiles: custom-DVE ops (reciprocal) need partition
            # base 0, so each q-block gets its own 4-row tile (row = head)
            sums_qb = [
                glob.tile([4, QBW], F32, tag="sums", name=f"sums{i}")
                for i in range(NQB)
            ]
            rcp_f32 = glob.tile([4, QBW], F32, tag="rcp_f32")
            rcp_bf = glob.tile([4, QBW], BF16, tag="rcp_bf")
            rcp_scr = glob.tile([4, QBW], F32, tag="rcp_scr")
            ident = glob.tile([128, 128], F32, tag="ident")
            warm = glob.tile([1, 16], F32, tag="warm")

            nc.vector.memset(v_s[:, :, 64], 1.0)
            for t in sums_qb:
                nc.vector.memset(t[:], 1.0)
            # Preload the Exp table set during the initial DMA wait.
            nc.vector.memset(warm[:], 0.0)
            nc.scalar.activation(warm[:], warm[:], AF.Exp)

            with (
                tc.tile_pool(name="p1", bufs=1) as p1,
                tc.tile_pool(name="xp", bufs=3) as xp,
                tc.tile_pool(name="tmpp", bufs=4) as tmpp,
                tc.tile_pool(name="qsp", bufs=4) as qsp,
                tc.tile_pool(name="ptp", bufs=8) as ptp,
                tc.tile_pool(name="stgp", bufs=4) as stgp,
                tc.tile_pool(name="yp", bufs=8) as yp,
                tc.tile_pool(name="ps1", bufs=2, space="PSUM") as ps1,
                tc.tile_pool(name="pssc", bufs=2, space="PSUM") as pssc,
                tc.tile_pool(name="pso_p", bufs=2, space="PSUM") as pso_p,
            ):
                wq_s = p1.tile([128, DCH, 384], BF16, tag="wq_s")
                wqkv_r = wqkv_d.rearrange("(ko p) n -> p ko n", p=128)
                cos_s = p1.tile([128, S], BF16, tag="cos_s")
                sin_s = p1.tile([128, S], BF16, tag="sin_s")
                kvraw = p1.tile([128, S], F32, tag="kvraw")
                kswap = p1.tile([64, S], F32, tag="kswap")

                qs_all = [
                    qsp.tile([128, S], BF16, tag="qs", name=f"qs{i}") for i in range(HQ)
                ]

                def stream_qs(h, qb):
                    """Stage head-h q data (duplicated per array-half) for block qb."""
                    hc = slice(32 * h, 32 * h + 32)
                    qc = slice(qb * QBW, (qb + 1) * QBW)
                    qs = qs_all[h]
                    nc.sync.dma_start(qs[0:32, qc], outA[hc, qc])
                    nc.sync.dma_start(qs[32:64, qc], outB[hc, qc])
                    nc.sync.dma_start(qs[64:96, qc], outA[hc, qc])
                    nc.sync.dma_start(qs[96:128, qc], outB[hc, qc])

                def unit_gen(h, qb):
                    """Scores + softmax-exp + PV for (head h, q-block qb),
                    yielding after each chunk-pair so the caller can interleave
                    pair emission with projection-chain segments (keeps the
                    exp pipeline fed while long PE chains run).

                    Diagonal chunks (kc0 >= q0) only compute/consume the causal
                    q-range [kc0, q0+512): d = kc0-q0 cols are skipped in the
                    scores MM, exp, select and PV.  The one exception: when
                    d == 128 (second chunk of the second-to-last pair) the
                    scores MM computes from 0 anyway so a single exp can span
                    [dA:1024] without reading unwritten PSUM; the extra cols
                    are never consumed downstream.
                    """
                    qs = qs_all[h]
                    q0 = qb * QBW
                    nkc = 4 * (qb + 1)
                    pso = pso_p.tile([128, QBW], F32, tag="pso")
                    for pair in range(nkc // 2):
                        yield
                        cA, cB = 2 * pair, 2 * pair + 1
                        psc = pssc.tile([128, 1024], F32, tag="psc")
                        ptt = ptp.tile([128, 1024], BF16, tag="ptt")
                        dA = max(0, cA * 128 - q0)
                        dB = max(0, cB * 128 - q0)
                        for c, half, r0, d in ((cA, 0, 0, dA), (cB, 1, 64, dB)):
                            dm = 0 if d == 128 else d
                            nc.tensor.matmul(
                                psc[:, half * 512 + dm : half * 512 + 512],
                                lhsT=ktdup[r0 : r0 + 64, c * 128 : c * 128 + 128],
                                rhs=qs[r0 : r0 + 64, q0 + dm : q0 + QBW],
                                start=True,
                                stop=True,
                                tile_position=(r0, 0),
                            )
                        if dB > 128:
                            nc.scalar.activation(
                                ptt[:, dA:512], psc[:, dA:512], AF.Exp
                            )
                            nc.scalar.activation(
                                ptt[:, 512 + dB : 1024], psc[:, 512 + dB : 1024],
                                AF.Exp,
                            )
                        else:
                            nc.scalar.activation(
                                ptt[:, dA:1024], psc[:, dA:1024], AF.Exp
                            )
                        for c, half, d in ((cA, 0, dA), (cB, 1, dB)):
                            kc0 = c * 128
                            if kc0 >= q0:
                                ww = min(128, 512 - d)
                                s0 = half * 512 + d
                                nc.gpsimd.affine_select(
                                    out=ptt[:, s0 : s0 + ww],
                                    in_=ptt[:, s0 : s0 + ww],
                                    compare_op=ALU.is_ge,
                                    fill=0.0,
                                    base=0,
                                    channel_multiplier=-1,
                                    pattern=[[1, ww]],
                                )
                        for c, half, d in ((cA, 0, dA), (cB, 1, dB)):
                            nc.tensor.matmul(
                                pso[0:65, d:QBW],
                                lhsT=v_s[:, c, :],
                                rhs=ptt[:, half * 512 + d : half * 512 + 512],
                                start=(c == 0),
                                stop=(c == nkc - 1),
                            )
                    # evict raw attn out (rows 0-63) + exp-sum (row 64)
                    ch = h // 2
                    rr = 64 * (h % 2)
                    qc = slice(q0, q0 + QBW)
                    if rr == 0:
                        nc.vector.tensor_copy(ao[0:64, ch, qc], pso[0:64, :])
                    else:
                        stg = stgp.tile([64, QBW], BF16, tag="stg")
                        nc.vector.tensor_copy(stg[:], pso[0:64, :])
                        nc.sync.dma_start(ao[64:128, ch, qc], stg[:])
                    sumr = stgp.tile([1, QBW], F32, tag="sumr")
                    nc.vector.tensor_copy(sumr[:], pso[64:65, :])
                    nc.gpsimd.dma_start(sums_qb[qb][h : h + 1, :], sumr[:])

                pending = []  # FIFO of [qb, unit generator]
                _rr = [0]

                def pump(n):
                    """Advance up to n pair-steps, round-robin over the two
                    oldest pending units (two independent score/exp/PV streams
                    hide each other's latency)."""
                    done = 0
                    while pending and done < n:
                        k = _rr[0] % min(2, len(pending))
                        try:
                            next(pending[k][1])
                            done += 1
                            _rr[0] += 1
                        except StopIteration:
                            pending.pop(k)

                def drain_qb(qb):
                    """Fully emit every pending unit of q-blocks <= qb."""
                    i = 0
                    while i < len(pending):
                        if pending[i][0] <= qb:
                            for _ in pending[i][1]:
                                pass
                            pending.pop(i)
                        else:
                            i += 1

                def norm_oproj(qb):
                    """Normalize all heads for q-block qb, then o_proj its rows."""
                    q0 = qb * QBW
                    qc = slice(q0, q0 + QBW)
                    nc.vector.reciprocal_approx_accurate(
                        rcp_f32[:], sums_qb[qb][:], rcp_scr[:]
                    )
                    nc.vector.tensor_copy(rcp_bf[:], rcp_f32[:])
                    for ch in range(2):
                        pbc = ps1.tile([128, QBW], F32, tag="proj")
                        nc.tensor.matmul(
                            pbc[:],
                            lhsT=sel_s[:, ch * 128 : (ch + 1) * 128],
                            rhs=rcp_bf[:],
                            start=True,
                            stop=True,
                        )
                        nc.vector.tensor_tensor(
                            ao[:, ch, qc], ao[:, ch, qc], pbc[:], ALU.mult
                        )
                    for st in range(4 * qb, 4 * qb + 4):
                        for obp in range(2):
                            if qb == 3 and obp == 1:
                                # exp stream is over: use the freed score-PSUM
                                # banks to deepen the o_proj pipeline
                                psys = [
                                    pssc.tile(
                                        [128, 1024], F32, tag="psc", name=f"psy{oh}"
                                    )[:, 0:QBW]
                                    for oh in range(2)
                                ]
                            else:
                                psys = [
                                    ps1.tile(
                                        [128, QBW], F32, tag="proj", name=f"psy{oh}"
                                    )
                                    for oh in range(2)
                                ]
                            for chp in range(2):
                                for oh in range(2):
                                    ob = 2 * obp + oh
                                    nc.tensor.matmul(
                                        psys[oh][:],
                                        lhsT=ao[:, chp, st * 128 : (st + 1) * 128],
                                        rhs=wo_s[:, chp, ob * 512 : (ob + 1) * 512],
                                        start=(chp == 0),
                                        stop=(chp == 1),
                                    )
                            for oh in range(2):
                                ob = 2 * obp + oh
                                ysb = yp.tile([128, QBW], BF16, tag="ysb")
                                if qb == 3 and oh == 1:
                                    nc.scalar.activation(ysb[:], psys[oh][:], AF.Copy)
                                else:
                                    nc.vector.tensor_copy(ysb[:], psys[oh][:])
                                eng = nc.gpsimd if (st + ob) % 2 == 0 else nc.sync
                                eng.dma_start(
                                    y_d[
                                        st * 128 : (st + 1) * 128,
                                        ob * 512 : (ob + 1) * 512,
                                    ],
                                    ysb[:],
                                )

                # units become available after projection s-block sb (unit
                # (h, qb) needs s-blocks 0..qb); their chunk-pairs are pumped
                # one at a time between projection-chain segments
                unit_sched = {
                    0: [(0, 0), (1, 0), (2, 0), (3, 0)],
                    1: [(0, 1), (1, 1), (2, 1), (3, 1)],
                    2: [(0, 2), (1, 2), (2, 2), (3, 2)],
                    3: [(0, 3), (1, 3), (2, 3), (3, 3)],
                }

                xt_r = xt_d.rearrange("(ko p) s -> p ko s", p=128)
                for sb in range(NSB):
                    sbc = slice(sb * SBW, (sb + 1) * SBW)
                    xblk = xp.tile([128, DCH, SBW], BF16, tag="xblk")
                    for kq in range(4):
                        if sb == 0:
                            # weights stream on the scalar HWDGE queue: it is
                            # otherwise dead until the first exp (~25us), and
                            # this overlaps the weight load with the x blocks
                            for kc in range(4 * kq, 4 * kq + 4):
                                nc.scalar.dma_start(wq_s[:, kc, :], wqkv_r[:, kc, :])
                        nc.sync.dma_start(
                            xblk[:, 4 * kq : 4 * kq + 4, :],
                            xt_r[:, 4 * kq : 4 * kq + 4, sbc],
                        )
                    if sb == 0:
                        nc.scalar.dma_start(cos_s[:], cos_d[:])
                        nc.scalar.dma_start(sin_s[:], sin_d[:])
                        make_identity(nc, ident[:])
                    if sb == 1:
                        nc.gpsimd.dma_start(sel_s[:], sel_d[:])
                        for ch in range(2):
                            nc.gpsimd.dma_start(wo_s[:, ch, :], wo_d[ch])
                    psKV = ps1.tile([128, SBW], F32, tag="proj")
                    psA = ps1.tile([128, SBW], F32, tag="proj")
                    psB = ps1.tile([128, SBW], F32, tag="proj")
                    for ps_t, col0 in ((psKV, 256), (psA, 0), (psB, 128)):
                        for kq in range(4):
                            for kc in range(4 * kq, 4 * kq + 4):
                                nc.tensor.matmul(
                                    ps_t[:],
                                    lhsT=wq_s[:, kc, col0 : col0 + 128],
                                    rhs=xblk[:, kc, :],
                                    start=(kc == 0),
                                    stop=(kc == DCH - 1),
                                )
                            pump(1)
                    # evict k|v rows early (frees the KV slot); DVE keeps the
                    # scalar engine free for the exp stream
                    nc.vector.tensor_copy(kvraw[:, sbc], psKV[:])

                    # RoPE on the 4 q heads (A = first-half dims, B = second).
                    # psA's two reads come first so its ps1 slot frees after
                    # just two DVE ops (psB after four).
                    tmp = tmpp.tile([128, SBW], F32, tag="tmp")
                    tmp2 = tmpp.tile([128, SBW], F32, tag="tmp")
                    nc.vector.tensor_tensor(
                        outA[:, sbc], psA[:], cos_s[:, sbc], ALU.mult
                    )
                    nc.vector.tensor_tensor(tmp2[:], psA[:], sin_s[:, sbc], ALU.mult)
                    nc.vector.tensor_tensor(tmp[:], psB[:], sin_s[:, sbc], ALU.mult)
                    nc.vector.tensor_tensor(
                        outB[:, sbc], psB[:], cos_s[:, sbc], ALU.mult
                    )
                    nc.vector.tensor_tensor(
                        outA[:, sbc], outA[:, sbc], tmp[:], ALU.subtract
                    )
                    nc.vector.tensor_tensor(
                        outB[:, sbc], outB[:, sbc], tmp2[:], ALU.add
                    )

                    # k RoPE on this s-block: kswap = [k_hi; k_lo]
                    nc.gpsimd.dma_start(kswap[0:32, sbc], kvraw[32:64, sbc])
                    nc.gpsimd.dma_start(kswap[32:64, sbc], kvraw[0:32, sbc])
                    nc.vector.tensor_tensor(
                        ktdup[0:64, sbc], kvraw[0:64, sbc], cos_s[0:64, sbc], ALU.mult
                    )
                    tmpk = tmpp.tile([64, SBW], F32, tag="tmpk")
                    nc.vector.tensor_tensor(
                        tmpk[:], kswap[:, sbc], sin_s[0:64, sbc], ALU.mult
                    )
                    nc.vector.tensor_tensor(
                        ktdup[0:32, sbc], ktdup[0:32, sbc], tmpk[0:32, :],
                        ALU.subtract,
                    )
                    nc.vector.tensor_tensor(
                        ktdup[32:64, sbc], ktdup[32:64, sbc], tmpk[32:64, :],
                        ALU.add,
                    )
                    nc.gpsimd.dma_start(ktdup[64:128, sbc], ktdup[0:64, sbc])

                    # v: [64, 512] -> 4 key-chunk tiles [128, 64] via PE transpose
                    for c in range(4 * sb, 4 * sb + 4):
                        ptr = pso_p.tile([128, QBW], F32, tag="pso")
                        nc.tensor.transpose(
                            ptr[:, 0:64],
                            kvraw[64:128, c * 128 : (c + 1) * 128],
                            ident[64:128, 64:128],
                        )
                        nc.vector.tensor_copy(v_s[:, c, 0:64], ptr[:, 0:64])
                        pump(1)

                    for h, qb in unit_sched[sb]:
                        stream_qs(h, qb)
                        pending.append([qb, unit_gen(h, qb)])
                    pump(8 if sb == 0 else 3)

                # tail: remaining unit pairs interleaved with per-block
                # norm + o_proj (norm_oproj(qb) only needs units (*, qb), so
                # its PE work overlaps the still-streaming exp pipeline)
                for qb in range(NQB):
                    drain_qb(qb)
                    norm_oproj(qb)
                    pump(6)
    nc.compile()
    return nc


def _prep_inputs(x, Wq, Wk, Wv, Wo, inv_freq):
    """Host-side sharding + layout prep. Returns in_maps for the 8 cores."""
    x = np.ascontiguousarray(np.asarray(x, dtype=np.float32).reshape(S, D))
    xt = np.ascontiguousarray(x.T)  # [D, S]

    pos = np.arange(S, dtype=np.float64)
    inv = np.asarray(inv_freq, dtype=np.float64)  # [32]
    freqs = pos[None, :] * inv[:, None]  # [32, S]
    cos32 = np.cos(freqs).astype(np.float32)
    sin32 = np.sin(freqs).astype(np.float32)
    cos_tab = np.tile(cos32, (4, 1))  # [128, S]
    sin_tab = np.tile(sin32, (4, 1))
    # sel[h, ch*128 + r]: broadcast rcp row (head) 2ch into output rows 0-63
    # and 2ch+1 into rows 64-127 (same pattern for every q-block).
    sel = np.zeros((4, 2 * 128), dtype=np.float32)
    for ch in range(2):
        sel[2 * ch, ch * 128 : ch * 128 + 64] = 1.0
        sel[2 * ch + 1, ch * 128 + 64 : ch * 128 + 128] = 1.0

    in_maps = []
    for i in range(NCORES):
        wq_l = Wq[256 * i : 256 * (i + 1)].astype(np.float32) * 0.125  # [256, D]
        wk_l = Wk[64 * i : 64 * (i + 1)].astype(np.float32)  # [64, D]
        wv_l = Wv[64 * i : 64 * (i + 1)].astype(np.float32)  # [64, D]
        # A-tile: first-half dims of the 4 heads; B-tile: second halves
        wA = np.concatenate(
            [wq_l[64 * h : 64 * h + 32] for h in range(HQ)], axis=0
        )  # [128, D]
        wB = np.concatenate(
            [wq_l[64 * h + 32 : 64 * h + 64] for h in range(HQ)], axis=0
        )
        wkv = np.concatenate([wk_l, wv_l], axis=0)  # [128, D]
        wqkv = np.ascontiguousarray(
            np.concatenate([wA, wB, wkv], axis=0).T
        )  # [D, 384]
        wo_l = Wo[:, 256 * i : 256 * (i + 1)].astype(np.float32)  # [D, 256]
        wo_t = np.ascontiguousarray(wo_l.T.reshape(2, 128, D))  # [2, 128, D]
        in_maps.append(
            {
                "xt": xt.astype(ml_dtypes.bfloat16),
                "wqkv": wqkv.astype(ml_dtypes.bfloat16),
                "wo": wo_t.astype(ml_dtypes.bfloat16),
                "cos": cos_tab.astype(ml_dtypes.bfloat16),
                "sin": sin_tab.astype(ml_dtypes.bfloat16),
                "sel": sel.astype(ml_dtypes.bfloat16),
            }
        )
    return in_maps


_NC_CACHE = None


def kernel(x, Wq, Wk, Wv, Wo, inv_freq):
    global _NC_CACHE
    if _NC_CACHE is None:
        _NC_CACHE = _build_nc()
    nc = _NC_CACHE
    in_maps = _prep_inputs(x, Wq, Wk, Wv, Wo, inv_freq)
    trace = bool(int(os.environ.get("BASS_KERNEL_TRACE", "0")))
    res = None
    last_exc = None
    for attempt in range(3):
        try:
            res = run_bass_kernel_spmd(nc, in_maps, list(range(NCORES)), trace=trace)
            break
        except Exception as e:  # transient device faults (rare) — retry
            last_exc = e
            msg = str(e)
            if "UNRECOVERABLE" in msg or "UNAVAILABLE" in msg or "Timeout" in msg:
                continue
            raise
    if res is None:
        raise last_exc
    if trace:
        kernel.last_results = res
    y = np.zeros((S, D), dtype=np.float32)
    for i in range(NCORES):
        y += res.results[i]["y"].astype(np.float32)
    return y.reshape(1, S, D)


# revision 44
# speedup vs baseline: 1.2007x; 1.1432x over previous
"""Trainium2 Bass kernel for GQA attention (B=1, S=2048, D=2048, H=32, KV=8, HD=64).

Tensor-parallel over heads across 8 NeuronCores: core i holds q-heads
[4i, 4i+4) and kv-head i; each core computes its partial o_proj output and the
host sums the 8 partials (Megatron all-reduce done host-side).

Schedule: attention units (head h, q-block qb) are interleaved into the
projection phase as soon as their inputs exist (unit (h,qb) only needs
s-blocks 0..qb projected), so the ACT engine (exp is the per-unit bottleneck)
is busy from early on while the PE does projections. o_proj is released
per-q-block as soon as all 4 heads of that block are normalized.

Self-contained: only imports concourse (on sys.path in the container).
"""

import os
import sys

import ml_dtypes
import numpy as np

if "/opt/trn_rl_repo" not in sys.path and not any(
    p.endswith("trn_rl_repo") for p in sys.path
):
    sys.path.insert(0, "/opt/trn_rl_repo")

import concourse.bass as bass
import concourse.mybir as mybir
import concourse.tile as tile
from concourse import bacc
from concourse.bass_utils import run_bass_kernel_spmd
from concourse.masks import make_identity

F32 = mybir.dt.float32
BF16 = mybir.dt.bfloat16

AF = mybir.ActivationFunctionType
ALU = mybir.AluOpType

S = 2048
D = 2048
H = 32
KV = 8
HD = 64
NCORES = 8
HQ = H // NCORES  # 4 q heads per core
NKC = S // 128  # 16 key chunks
NQB = 4  # q blocks of 512
QBW = 512
NSB = 4  # s blocks of 512 in projection
SBW = 512
DCH = D // 128  # 16 contraction chunks


def _build_nc():
    nc = bacc.Bacc("TRN2", target_bir_lowering=False, debug=False, num_devices=NCORES)

    xt_d = nc.declare_dram_parameter("xt", [D, S], BF16, isOutput=False)
    wqkv_d = nc.declare_dram_parameter("wqkv", [D, 384], BF16, isOutput=False)
    wo_d = nc.declare_dram_parameter("wo", [2, 128, D], BF16, isOutput=False)
    cos_d = nc.declare_dram_parameter("cos", [128, S], BF16, isOutput=False)
    sin_d = nc.declare_dram_parameter("sin", [128, S], BF16, isOutput=False)
    sel_d = nc.declare_dram_parameter("sel", [4, 2 * 128], BF16, isOutput=False)
    y_d = nc.declare_dram_parameter("y", [S, D], BF16, isOutput=True)

    with tile.TileContext(nc) as tc:
        with (
            tc.tile_pool(name="glob", bufs=1) as glob,
        ):
            ktdup = glob.tile([128, S], BF16, tag="ktdup")
            v_s = glob.tile([128, NKC, 65], BF16, tag="v_s")
            outA = glob.tile([128, S], BF16, tag="outA")
            outB = glob.tile([128, S], BF16, tag="outB")
            ao = glob.tile([128, 2, S], BF16, tag="ao")
            sel_s = glob.tile([4, 2 * 128], BF16, tag="sel_s")
            wo_s = glob.tile([128, 2, D], BF16, tag="wo_s")
            # per-qb sum tiles: custom-DVE ops (reciprocal) need partition
            # base 0, so each q-block gets its own 4-row tile (row = head)
            sums_qb = [
                glob.tile([4, QBW], F32, tag="sums", name=f"sums{i}")
                for i in range(NQB)
            ]
            rcp_f32 = glob.tile([4, QBW], F32, tag="rcp_f32")
            rcp_bf = glob.tile([4, QBW], BF16, tag="rcp_bf")
            rcp_scr = glob.tile([4, QBW], F32, tag="rcp_scr")
            ident = glob.tile([128, 128], F32, tag="ident")
            warm = glob.tile([1, 16], F32, tag="warm")

            nc.vector.memset(v_s[:, :, 64], 1.0)
            for t in sums_qb:
                nc.vector.memset(t[:], 1.0)
            # Preload the Exp table set during the initial DMA wait.
            nc.vector.memset(warm[:], 0.0)
            nc.scalar.activation(warm[:], warm[:], AF.Exp)

            with (
                tc.tile_pool(name="p1", bufs=1) as p1,
                tc.tile_pool(name="xp", bufs=3) as xp,
                tc.tile_pool(name="tmpp", bufs=4) as tmpp,
                tc.tile_pool(name="qsp", bufs=4) as qsp,
                tc.tile_pool(name="ptp", bufs=8) as ptp,
                tc.tile_pool(name="stgp", bufs=4) as stgp,
                tc.tile_pool(name="yp", bufs=8) as yp,
                tc.tile_pool(name="ps1", bufs=2, space="PSUM") as ps1,
                tc.tile_pool(name="pssc", bufs=2, space="PSUM") as pssc,
                tc.tile_pool(name="pso_p", bufs=2, space="PSUM") as pso_p,
            ):
                wq_s = p1.tile([128, DCH, 384], BF16, tag="wq_s")
                wqkv_r = wqkv_d.rearrange("(ko p) n -> p ko n", p=128)
                cos_s = p1.tile([128, S], BF16, tag="cos_s")
                sin_s = p1.tile([128, S], BF16, tag="sin_s")
                kvraw = p1.tile([128, S], F32, tag="kvraw")
                kswap = p1.tile([64, S], F32, tag="kswap")

                qs_all = [
                    qsp.tile([128, S], BF16, tag="qs", name=f"qs{i}") for i in range(HQ)
                ]

                def stream_qs(h, qb):
                    """Stage head-h q data (duplicated per array-half) for block qb."""
                    hc = slice(32 * h, 32 * h + 32)
                    qc = slice(qb * QBW, (qb + 1) * QBW)
                    qs = qs_all[h]
                    nc.sync.dma_start(qs[0:32, qc], outA[hc, qc])
                    nc.sync.dma_start(qs[32:64, qc], outB[hc, qc])
                    nc.sync.dma_start(qs[64:96, qc], outA[hc, qc])
                    nc.sync.dma_start(qs[96:128, qc], outB[hc, qc])

                def unit_gen(h, qb):
                    """Scores + softmax-exp + PV for (head h, q-block qb),
                    yielding after each chunk-pair so the caller can interleave
                    pair emission with projection-chain segments (keeps the
                    exp pipeline fed while long PE chains run).

                    Diagonal chunks (kc0 >= q0) only compute/consume the causal
                    q-range [kc0, q0+512): d = kc0-q0 cols are skipped in the
                    scores MM, exp, select and PV.  The one exception: when
                    d == 128 (second chunk of the second-to-last pair) the
                    scores MM computes from 0 anyway so a single exp can span
                    [dA:1024] without reading unwritten PSUM; the extra cols
                    are never consumed downstream.
                    """
                    qs = qs_all[h]
                    q0 = qb * QBW
                    nkc = 4 * (qb + 1)
                    pso = pso_p.tile([128, QBW], F32, tag="pso")
                    for pair in range(nkc // 2):
                        yield
                        cA, cB = 2 * pair, 2 * pair + 1
                        psc = pssc.tile([128, 1024], F32, tag="psc")
                        ptt = ptp.tile([128, 1024], BF16, tag="ptt")
                        dA = max(0, cA * 128 - q0)
                        dB = max(0, cB * 128 - q0)
                        for c, half, r0, d in ((cA, 0, 0, dA), (cB, 1, 64, dB)):
                            dm = 0 if d == 128 else d
                            nc.tensor.matmul(
                                psc[:, half * 512 + dm : half * 512 + 512],
                                lhsT=ktdup[r0 : r0 + 64, c * 128 : c * 128 + 128],
                                rhs=qs[r0 : r0 + 64, q0 + dm : q0 + QBW],
                                start=True,
                                stop=True,
                                tile_position=(r0, 0),
                            )
                        if dB > 128:
                            nc.scalar.activation(
                                ptt[:, dA:512], psc[:, dA:512], AF.Exp
                            )
                            nc.scalar.activation(
                                ptt[:, 512 + dB : 1024], psc[:, 512 + dB : 1024],
                                AF.Exp,
                            )
                        else:
                            nc.scalar.activation(
                                ptt[:, dA:1024], psc[:, dA:1024], AF.Exp
                            )
                        for c, half, d in ((cA, 0, dA), (cB, 1, dB)):
                            kc0 = c * 128
                            if kc0 >= q0:
                                ww = min(128, 512 - d)
                                s0 = half * 512 + d
                                nc.gpsimd.affine_select(
                                    out=ptt[:, s0 : s0 + ww],
                                    in_=ptt[:, s0 : s0 + ww],
                                    compare_op=ALU.is_ge,
                                    fill=0.0,
                                    base=0,
                                    channel_multiplier=-1,
                                    pattern=[[1, ww]],
                                )
                        for c, half, d in ((cA, 0, dA), (cB, 1, dB)):
                            nc.tensor.matmul(
                                pso[0:65, d:QBW],
                                lhsT=v_s[:, c, :],
                                rhs=ptt[:, half * 512 + d : half * 512 + 512],
                                start=(c == 0),
                                stop=(c == nkc - 1),
                            )
                    # evict raw attn out (rows 0-63) + exp-sum (row 64)
                    ch = h // 2
                    rr = 64 * (h % 2)
                    qc = slice(q0, q0 + QBW)
                    if rr == 0:
                        nc.vector.tensor_copy(ao[0:64, ch, qc], pso[0:64, :])
                    else:
                        stg = stgp.tile([64, QBW], BF16, tag="stg")
                        nc.vector.tensor_copy(stg[:], pso[0:64, :])
                        nc.sync.dma_start(ao[64:128, ch, qc], stg[:])
                    sumr = stgp.tile([1, QBW], F32, tag="sumr")
                    nc.vector.tensor_copy(sumr[:], pso[64:65, :])
                    nc.gpsimd.dma_start(sums_qb[qb][h : h + 1, :], sumr[:])

                pending = []  # FIFO of [qb, unit generator]
                _rr = [0]

                def pump(n):
                    """Advance up to n pair-steps, round-robin over the two
                    oldest pending units (two independent score/exp/PV streams
                    hide each other's latency)."""
                    done = 0
                    while pending and done < n:
                        k = _rr[0] % min(2, len(pending))
                        try:
                            next(pending[k][1])
                            done += 1
                            _rr[0] += 1
                        except StopIteration:
                            pending.pop(k)

                def drain_qb(qb):
                    """Fully emit every pending unit of q-blocks <= qb."""
                    i = 0
                    while i < len(pending):
                        if pending[i][0] <= qb:
                            for _ in pending[i][1]:
                                pass
                            pending.pop(i)
                        else:
                            i += 1

                def norm_oproj(qb):
                    """Normalize all heads for q-block qb, then o_proj its rows."""
                    q0 = qb * QBW
                    qc = slice(q0, q0 + QBW)
                    nc.vector.reciprocal_approx_accurate(
                        rcp_f32[:], sums_qb[qb][:], rcp_scr[:]
                    )
                    nc.vector.tensor_copy(rcp_bf[:], rcp_f32[:])
                    for ch in range(2):
                        pbc = ps1.tile([128, QBW], F32, tag="proj")
                        nc.tensor.matmul(
                            pbc[:],
                            lhsT=sel_s[:, ch * 128 : (ch + 1) * 128],
                            rhs=rcp_bf[:],
                            start=True,
                            stop=True,
                        )
                        nc.vector.tensor_tensor(
                            ao[:, ch, qc], ao[:, ch, qc], pbc[:], ALU.mult
                        )
                    for st in range(4 * qb, 4 * qb + 4):
                        for obp in range(2):
                            if qb == 3 and obp == 1:
                                # exp stream is over: use the freed score-PSUM
                                # banks to deepen the o_proj pipeline
                                psys = [
                                    pssc.tile(
                                        [128, 1024], F32, tag="psc", name=f"psy{oh}"
                                    )[:, 0:QBW]
                                    for oh in range(2)
                                ]
                            else:
                                psys = [
                                    ps1.tile(
                                        [128, QBW], F32, tag="proj", name=f"psy{oh}"
                                    )
                                    for oh in range(2)
                                ]
                            for chp in range(2):
                                for oh in range(2):
                                    ob = 2 * obp + oh
                                    nc.tensor.matmul(
                                        psys[oh][:],
                                        lhsT=ao[:, chp, st * 128 : (st + 1) * 128],
                                        rhs=wo_s[:, chp, ob * 512 : (ob + 1) * 512],
                                        start=(chp == 0),
                                        stop=(chp == 1),
                                    )
                            for oh in range(2):
                                ob = 2 * obp + oh
                                ysb = yp.tile([128, QBW], BF16, tag="ysb")
                                if qb == 3 and oh == 1:
                                    nc.scalar.activation(ysb[:], psys[oh][:], AF.Copy)
                                else:
                                    nc.vector.tensor_copy(ysb[:], psys[oh][:])
                                eng = nc.gpsimd if (st + ob) % 2 == 0 else nc.sync
                                eng.dma_start(
                                    y_d[
                                        st * 128 : (st + 1) * 128,
                                        ob * 512 : (ob + 1) * 512,
                                    ],
                                    ysb[:],
                                )
                        # keep score-pairs flowing to the exp engine between
                        # o_proj tiles (else o_proj hogs PE and ACT starves)
                        pump(2)

                # units become available after projection s-block sb (unit
                # (h, qb) needs s-blocks 0..qb); their chunk-pairs are pumped
                # one at a time between projection-chain segments
                unit_sched = {
                    0: [(0, 0), (1, 0), (2, 0), (3, 0)],
                    1: [(0, 1), (1, 1), (2, 1), (3, 1)],
                    2: [(0, 2), (1, 2), (2, 2), (3, 2)],
                    3: [(0, 3), (1, 3), (2, 3), (3, 3)],
                }

                xt_r = xt_d.rearrange("(ko p) s -> p ko s", p=128)
                for sb in range(NSB):
                    sbc = slice(sb * SBW, (sb + 1) * SBW)
                    xblk = xp.tile([128, DCH, SBW], BF16, tag="xblk")
                    for kq in range(4):
                        if sb == 0:
                            # weights stream on the scalar HWDGE queue: it is
                            # otherwise dead until the first exp (~25us), and
                            # this overlaps the weight load with the x blocks
                            for kc in range(4 * kq, 4 * kq + 4):
                                nc.scalar.dma_start(wq_s[:, kc, :], wqkv_r[:, kc, :])
                        nc.sync.dma_start(
                            xblk[:, 4 * kq : 4 * kq + 4, :],
                            xt_r[:, 4 * kq : 4 * kq + 4, sbc],
                        )
                    if sb == 0:
                        nc.scalar.dma_start(cos_s[:], cos_d[:])
                        nc.scalar.dma_start(sin_s[:], sin_d[:])
                        make_identity(nc, ident[:])
                    if sb == 1:
                        nc.gpsimd.dma_start(sel_s[:], sel_d[:])
                        for ch in range(2):
                            nc.gpsimd.dma_start(wo_s[:, ch, :], wo_d[ch])
                    psKV = ps1.tile([128, SBW], F32, tag="proj")
                    psA = ps1.tile([128, SBW], F32, tag="proj")
                    psB = ps1.tile([128, SBW], F32, tag="proj")
                    for ps_t, col0 in ((psKV, 256), (psA, 0), (psB, 128)):
                        for kq in range(4):
                            for kc in range(4 * kq, 4 * kq + 4):
                                nc.tensor.matmul(
                                    ps_t[:],
                                    lhsT=wq_s[:, kc, col0 : col0 + 128],
                                    rhs=xblk[:, kc, :],
                                    start=(kc == 0),
                                    stop=(kc == DCH - 1),
                                )
                            pump(1)
                    # evict k|v rows early (frees the KV slot)
                    nc.scalar.activation(kvraw[:, sbc], psKV[:], AF.Copy)

                    # RoPE on the 4 q heads (A = first-half dims, B = second)
                    tmp = tmpp.tile([128, SBW], F32, tag="tmp")
                    nc.vector.tensor_tensor(
                        outA[:, sbc], psA[:], cos_s[:, sbc], ALU.mult
                    )
                    nc.vector.tensor_tensor(tmp[:], psB[:], sin_s[:, sbc], ALU.mult)
                    nc.vector.tensor_tensor(
                        outA[:, sbc], outA[:, sbc], tmp[:], ALU.subtract
                    )
                    tmp2 = tmpp.tile([128, SBW], F32, tag="tmp")
                    nc.vector.tensor_tensor(
                        outB[:, sbc], psB[:], cos_s[:, sbc], ALU.mult
                    )
                    nc.vector.tensor_tensor(tmp2[:], psA[:], sin_s[:, sbc], ALU.mult)
                    nc.vector.tensor_tensor(
                        outB[:, sbc], outB[:, sbc], tmp2[:], ALU.add
                    )

                    # k RoPE on this s-block: kswap = [k_hi; k_lo]
                    nc.gpsimd.dma_start(kswap[0:32, sbc], kvraw[32:64, sbc])
                    nc.gpsimd.dma_start(kswap[32:64, sbc], kvraw[0:32, sbc])
                    nc.vector.tensor_tensor(
                        ktdup[0:64, sbc], kvraw[0:64, sbc], cos_s[0:64, sbc], ALU.mult
                    )
                    tmpk = tmpp.tile([64, SBW], F32, tag="tmpk")
                    nc.vector.tensor_tensor(
                        tmpk[:], kswap[:, sbc], sin_s[0:64, sbc], ALU.mult
                    )
                    nc.vector.tensor_tensor(
                        ktdup[0:32, sbc], ktdup[0:32, sbc], tmpk[0:32, :],
                        ALU.subtract,
                    )
                    nc.vector.tensor_tensor(
                        ktdup[32:64, sbc], ktdup[32:64, sbc], tmpk[32:64, :],
                        ALU.add,
                    )
                    nc.gpsimd.dma_start(ktdup[64:128, sbc], ktdup[0:64, sbc])

                    # v: [64, 512] -> 4 key-chunk tiles [128, 64] via PE transpose
                    for c in range(4 * sb, 4 * sb + 4):
                        ptr = pso_p.tile([128, QBW], F32, tag="pso")
                        nc.tensor.transpose(
                            ptr[:, 0:64],
                            kvraw[64:128, c * 128 : (c + 1) * 128],
                            ident[64:128, 64:128],
                        )
                        nc.vector.tensor_copy(v_s[:, c, 0:64], ptr[:, 0:64])
                        pump(1)

                    for h, qb in unit_sched[sb]:
                        stream_qs(h, qb)
                        pending.append([qb, unit_gen(h, qb)])
                    pump(4 if sb == 0 else 2)

                # tail: remaining unit pairs interleaved with per-block
                # norm + o_proj (norm_oproj(qb) only needs units (*, qb), so
                # its PE work overlaps the still-streaming exp pipeline)
                for qb in range(NQB):
                    drain_qb(qb)
                    norm_oproj(qb)
                    pump(6)
    nc.compile()
    return nc


def _prep_inputs(x, Wq, Wk, Wv, Wo, inv_freq):
    """Host-side sharding + layout prep. Returns in_maps for the 8 cores."""
    x = np.ascontiguousarray(np.asarray(x, dtype=np.float32).reshape(S, D))
    xt = np.ascontiguousarray(x.T)  # [D, S]

    pos = np.arange(S, dtype=np.float64)
    inv = np.asarray(inv_freq, dtype=np.float64)  # [32]
    freqs = pos[None, :] * inv[:, None]  # [32, S]
    cos32 = np.cos(freqs).astype(np.float32)
    sin32 = np.sin(freqs).astype(np.float32)
    cos_tab = np.tile(cos32, (4, 1))  # [128, S]
    sin_tab = np.tile(sin32, (4, 1))
    # sel[h, ch*128 + r]: broadcast rcp row (head) 2ch into output rows 0-63
    # and 2ch+1 into rows 64-127 (same pattern for every q-block).
    sel = np.zeros((4, 2 * 128), dtype=np.float32)
    for ch in range(2):
        sel[2 * ch, ch * 128 : ch * 128 + 64] = 1.0
        sel[2 * ch + 1, ch * 128 + 64 : ch * 128 + 128] = 1.0

    in_maps = []
    for i in range(NCORES):
        wq_l = Wq[256 * i : 256 * (i + 1)].astype(np.float32) * 0.125  # [256, D]
        wk_l = Wk[64 * i : 64 * (i + 1)].astype(np.float32)  # [64, D]
        wv_l = Wv[64 * i : 64 * (i + 1)].astype(np.float32)  # [64, D]
        # A-tile: first-half dims of the 4 heads; B-tile: second halves
        wA = np.concatenate(
            [wq_l[64 * h : 64 * h + 32] for h in range(HQ)], axis=0
        )  # [128, D]
        wB = np.concatenate(
            [wq_l[64 * h + 32 : 64 * h + 64] for h in range(HQ)], axis=0
        )
        wkv = np.concatenate([wk_l, wv_l], axis=0)  # [128, D]
        wqkv = np.ascontiguousarray(
            np.concatenate([wA, wB, wkv], axis=0).T
        )  # [D, 384]
        wo_l = Wo[:, 256 * i : 256 * (i + 1)].astype(np.float32)  # [D, 256]
        wo_t = np.ascontiguousarray(wo_l.T.reshape(2, 128, D))  # [2, 128, D]
        in_maps.append(
            {
                "xt": xt.astype(ml_dtypes.bfloat16),
                "wqkv": wqkv.astype(ml_dtypes.bfloat16),
                "wo": wo_t.astype(ml_dtypes.bfloat16),
                "cos": cos_tab.astype(ml_dtypes.bfloat16),
                "sin": sin_tab.astype(ml_dtypes.bfloat16),
                "sel": sel.astype(ml_dtypes.bfloat16),
            }
        )
    return in_maps


_NC_CACHE = None


def kernel(x, Wq, Wk, Wv, Wo, inv_freq):
    global _NC_CACHE
    if _NC_CACHE is None:
        _NC_CACHE = _build_nc()
    nc = _NC_CACHE
    in_maps = _prep_inputs(x, Wq, Wk, Wv, Wo, inv_freq)
    trace = bool(int(os.environ.get("BASS_KERNEL_TRACE", "0")))
    res = None
    last_exc = None
    for attempt in range(3):
        try:
            res = run_bass_kernel_spmd(nc, in_maps, list(range(NCORES)), trace=trace)
            break
        except Exception as e:  # transient device faults (rare) — retry
            last_exc = e
            msg = str(e)
            if "UNRECOVERABLE" in msg or "UNAVAILABLE" in msg or "Timeout" in msg:
                continue
            raise
    if res is None:
        raise last_exc
    if trace:
        kernel.last_results = res
    y = np.zeros((S, D), dtype=np.float32)
    for i in range(NCORES):
        y += res.results[i]["y"].astype(np.float32)
    return y.reshape(1, S, D)
